# revision 31
# baseline (speedup 1.0000x reference)
"""GNN message-passing (scatter-mean + Linear) kernel for 8 Trainium2 NeuronCores.

reference:
    sums   = segment_sum(from_tensor, to_index, N)        # [N, 64]
    counts = segment_sum(ones, to_index, N)               # [N, 1]
    out    = (sums / max(counts, 1)) @ W.T + b            # [N, 64]

Sharding: edges are partitioned across the 8 cores BY DESTINATION NODE RANGE
(each core owns a contiguous block of N/8 nodes and receives the edges
targeting them), so no cross-core reduction is needed.

Device algorithm (per core): edges sorted by destination are cut into
128-edge chunks; a chunk touches at most S=8 distinct nodes (the rare chunk
that would touch more is cut early), so the chunk one-hot H[e, s] is only
[128, 8] — built on VectorE with a batched is_equal.  Chunks are processed
in PAIRS by a single TensorE matmul: the stationary is the two chunks'
features side by side ([128, 128] bf16) and the rhs is [H_A | H_B]
([128, 16]); the off-diagonal blocks of the [128, 16] PSUM output are
garbage and simply never read.  This halves the PE instruction count (the
real-HW bottleneck: per-matmul fixed cost + LdWeights).  64 chunks share a
[128, 512] PSUM bank; two strided ActE copies per bank extract chunk-A rows
(partitions 0:64) and chunk-B rows (partitions 64:128) into a shared SBUF
sums tile (bf16) with A on partitions 0:64 and B on 64:128.  Features are
PRESCALED on the host by 1/count[node], so slot sums are already means.
The final Linear runs per bank as two matmuls with W.T stationary at base
partition 0 (A) and 64 (B).  The bias is NOT applied on device: the host
gather computes out[n] = sum(slot rows of n) + b, which uniformly handles
normal nodes, nodes split across a chunk boundary, and empty nodes.
"""

import dataclasses

import ml_dtypes
import numpy as np

N_CORES = 8
P = 128          # SBUF partitions == edges per chunk == matmul contraction dim
S = 8            # one-hot width: max distinct nodes per chunk
HB = 32          # chunks per batched H build
TC = 128         # chunks per X-stream DMA tile
BK = 64          # chunks per PSUM bank (BK*S == 512 f32 == one 2KB bank)
PB = BK // 2     # pairs per bank
D = 64           # feature dim (in == out)

BF16 = ml_dtypes.bfloat16


def _pack_core(li):
    """Cut the sorted local node ids into 128-edge chunks, each touching at
    most S distinct nodes (cut early otherwise).  Returns (starts, ends,
    slot_of_edge, slot2node[NC, S])."""
    Ec = len(li)
    starts, ends, s2n = [], [], []
    slot_of_edge = np.empty(Ec, np.int64)
    pos = 0
    while pos < Ec:
        end = min(pos + P, Ec)
        seg = li[pos:end]
        u, first = np.unique(seg, return_index=True)
        if len(u) > S:
            end = pos + int(first[S])
            u = u[:S]
            seg = li[pos:end]
        slot_of_edge[pos:end] = np.searchsorted(u, seg)
        row = np.full(S, -1, np.int64)
        row[: len(u)] = u
        starts.append(pos)
        ends.append(end)
        s2n.append(row)
        pos = end
    return (
        np.asarray(starts),
        np.asarray(ends),
        slot_of_edge,
        np.asarray(s2n),
    )


def _prep_core(X, eid, li, slot_of_edge, starts, ends, NCp):
    """Build one core's device arrays: X_dev [P, NCp*D] bf16 (prescaled by
    1/count), li_dev [P, NCp] bf16 (slot ids)."""
    Ec = len(li)
    counts = np.bincount(li)
    recip = (1.0 / counts[li]).astype(np.float32)

    NCc = len(starts)
    chunk_of = np.repeat(np.arange(NCc), ends - starts)
    row = np.arange(Ec) - starts[chunk_of] + chunk_of * P

    Xg = np.zeros((NCp * P, D), np.float32)
    Xg[row] = X[eid] * recip[:, None]
    X_dev = np.ascontiguousarray(
        Xg.astype(BF16).reshape(NCp, P, D).transpose(1, 0, 2)
    ).reshape(P, NCp * D)

    lis = np.zeros(NCp * P, np.float32)
    lis[row] = slot_of_edge
    li_dev = np.ascontiguousarray(lis.reshape(NCp, P).T.astype(BF16))
    return X_dev, li_dev


def _bank_layout(NCp):
    """Full BK-chunk banks, with the trailing chunks split into small banks
    so the end-of-run extract->Linear->copy->flush chain is short (it sits
    entirely after the last X byte lands)."""
    rem = NCp - BK * max(0, (NCp - BK) // BK)
    bank_sizes = [BK] * ((NCp - rem) // BK)
    for t in (rem - 32, 16, 8, 8) if rem >= 64 else (rem,):
        if t > 0:
            bank_sizes.append(t)
    bank_cstart = [0]
    for bs in bank_sizes:
        bank_cstart.append(bank_cstart[-1] + bs)
    assert bank_cstart[-1] == NCp
    return bank_sizes, bank_cstart


def _build_bass(NCp):
    import concourse.bacc as bacc
    import concourse.mybir as mybir
    import concourse.tile as tile

    f32 = mybir.dt.float32
    bf16 = mybir.dt.bfloat16

    NP = NCp // 2           # chunk pairs
    HC = NCp * S // 2       # columns per half (A / B) of sums and out
    assert NCp % HB == 0
    bank_sizes, bank_cstart = _bank_layout(NCp)
    NB = len(bank_sizes)
    bank_of_chunk = np.repeat(np.arange(NB), bank_sizes)

    nc = bacc.Bacc("TRN2", target_bir_lowering=False)
    X_t = nc.dram_tensor("xdev", [P, NCp * D], bf16, kind="ExternalInput")
    li_t = nc.dram_tensor("lidev", [P, NCp], bf16, kind="ExternalInput")
    iota_t = nc.dram_tensor("iota", [P, S], bf16, kind="ExternalInput")
    w_t = nc.dram_tensor("wdup", [P, D], bf16, kind="ExternalInput")
    out_t = nc.dram_tensor("out", [D, NCp * S], bf16, kind="ExternalOutput")

    with tile.TileContext(nc) as tc:
        with (
            tc.tile_pool(name="const", bufs=1) as cp,
            tc.tile_pool(name="xin", bufs=7) as xp,
            tc.tile_pool(name="hp", bufs=4) as hp,
            tc.tile_pool(name="big", bufs=1) as bigp,
            tc.tile_pool(name="ps", bufs=4, space="PSUM") as pp,
            tc.tile_pool(name="ps2", bufs=4, space="PSUM") as pp2,
        ):
            # (tile schedule is needed below to place the lirel split safely)
            iota = cp.tile([P, S], bf16)
            nc.scalar.dma_start(out=iota[:], in_=iota_t[:, :])
            wdup = cp.tile([P, D], bf16)
            nc.scalar.dma_start(out=wdup[:], in_=w_t[:, :])

            # A-chunk sums on partitions 0:64, B-chunk sums on 64:128,
            # sharing columns (bank bi -> cols bi*256 .. bi*256+256)
            sums = bigp.tile([P, HC], bf16)
            outsb = bigp.tile([D, NCp * S], bf16)

            # moderate first/last tiles: keep per-partition DMA runs >= 4KB
            # (32 chunks) for full DMA rate while shrinking the PE backlog
            # that remains after the last X byte lands
            head = [32, 64] if NCp >= 256 else []
            tail = [64, 32, 32] if NCp >= 256 else []
            mid = NCp - sum(head) - sum(tail)
            sizes = (
                head
                + [TC] * (mid // TC)
                + ([mid % TC] if mid % TC else [])
                + tail
            )
            tiles = []
            base = 0
            for size in sizes:
                tiles.append((base, size))
                base += size
            assert base == NCp
            tile_of_chunk = {}
            for t, (b0, sz) in enumerate(tiles):
                for j in range(b0, b0 + sz):
                    tile_of_chunk[j] = t

            # lirel rides the fast sync queue AHEAD of the X tiles: the first
            # H build gates the whole PE pipeline start.  Only the slice the
            # first tile's H builds need goes up front; the rest is issued
            # right after the second tile's DMA (before any H build reads it).
            lsplit = tiles[1][0] if len(tiles) > 1 else NCp
            lirel = cp.tile([P, NCp], bf16)
            nc.sync.dma_start(out=lirel[:, :lsplit], in_=li_t[:, :lsplit])

            def emit_bank_extract(bi):
                # strided reads: pair p of this bank wrote [128, 16] at col
                # p*16; A slots are cols 0:8 (rows 0:64), B are 8:16 (64:128)
                np_ = bank_sizes[bi] // 2          # pairs in this bank
                c0_ = bank_cstart[bi] * S // 2     # sums col base
                tA = bank[0:64, :]
                inA = dataclasses.replace(
                    tA, ap=[tA.ap[0], [2 * S, np_], [1, S]]
                )
                outA = sums[0:64, c0_ : c0_ + np_ * S]
                nc.scalar.copy(
                    out=outA.rearrange("p (c w) -> p c w", w=S), in_=inA
                )
                tB = bank[64:128, S : np_ * 2 * S]
                inB = dataclasses.replace(
                    tB, ap=[tB.ap[0], [2 * S, np_], [1, S]]
                )
                outB = sums[64:128, c0_ : c0_ + np_ * S]
                nc.scalar.copy(
                    out=outB.rearrange("p (c w) -> p c w", w=S), in_=inB
                )

            def emit_final(bi):
                # per-bank Linear: A on partitions 0:64, B on 64:128; outsb
                # is bank-contiguous [A(w) | B(w)] so flushes are single DMAs
                w = bank_sizes[bi] * S // 2
                c0_ = bank_cstart[bi] * S // 2
                base = bank_cstart[bi] * S
                for half, p0 in ((0, 0), (1, 64)):
                    o2 = pp2.tile([D, w], f32)
                    nc.tensor.matmul(
                        o2[:],
                        lhsT=wdup[p0 : p0 + 64, :],
                        rhs=sums[p0 : p0 + 64, c0_ : c0_ + w],
                        start=True,
                        stop=True,
                    )
                    nc.scalar.copy(
                        out=outsb[:, base + half * w : base + (half + 1) * w],
                        in_=o2[:],
                    )

            def emit_out_dma(b0, b1, eng):
                c0_ = bank_cstart[b0] * S
                c1_ = bank_cstart[b1] * S
                eng.dma_start(
                    out=out_t[:, c0_:c1_], in_=outsb[:, c0_:c1_]
                )

            xt = h = bank = None
            xt_base = 0
            fin_done = 0
            dma_done = 0
            for q in range(NP):
                c0 = 2 * q
                t = tile_of_chunk[c0]
                if c0 == tiles[t][0]:
                    b0, sz = tiles[t]
                    if t == 1 and lsplit < NCp:
                        nc.sync.dma_start(
                            out=lirel[:, lsplit:], in_=li_t[:, lsplit:]
                        )
                    xt = xp.tile([P, TC * D], bf16, name="xt")
                    nc.sync.dma_start(
                        out=xt[:, : sz * D],
                        in_=X_t[:, b0 * D : (b0 + sz) * D],
                    )
                    xt_base = b0
                if c0 % HB == 0:
                    h = hp.tile([P, HB * S], bf16)
                    in0 = lirel[:, c0 : c0 + HB].to_broadcast([P, HB, S])
                    ia = iota[:, :]
                    in1 = dataclasses.replace(ia, ap=[ia.ap[0], [0, HB], [1, S]])
                    nc.vector.tensor_tensor(
                        out=h[:].rearrange("p (c w) -> p c w", w=S),
                        in0=in1,
                        in1=in0,
                        op=mybir.AluOpType.is_equal,
                    )
                bi = bank_of_chunk[c0]
                bq = (c0 - bank_cstart[bi]) // 2   # pair index within bank
                if bq == 0:
                    bank = pp.tile([P, bank_sizes[bi] * S], f32)
                nc.tensor.matmul(
                    bank[:, bq * 2 * S : (bq + 1) * 2 * S],
                    lhsT=xt[:, (c0 - xt_base) * D : (c0 - xt_base + 2) * D],
                    rhs=h[:, (c0 % HB) * S : ((c0 % HB) + 2) * S],
                    start=True,
                    stop=True,
                )
                if c0 == bank_cstart[bi + 1] - 2:
                    emit_bank_extract(bi)
                # run the Linear for bank bi one bank late so the PE never
                # head-of-line blocks on the ActE bank extraction
                if bq == min(8, bank_sizes[bi] // 4) and bi >= 1:
                    emit_final(fin_done)
                    fin_done += 1
                    if fin_done - dma_done >= 6:
                        emit_out_dma(dma_done, fin_done, nc.scalar)
                        dma_done = fin_done
            while fin_done < NB:
                emit_final(fin_done)
                fin_done += 1
            # tail flush rides the sync queue (full DMA rate, right after
            # the last X tile)
            emit_out_dma(dma_done, NB, nc.sync)
    nc.compile()
    return nc


_LAST_PERF = {}  # filled by kernel(): exec_time_ns etc (read by test.py)


def kernel(from_tensor, to_index, dim_size, W, b, _trace=False):
    from concourse.bass_utils import run_bass_kernel_spmd

    X = np.ascontiguousarray(np.asarray(from_tensor), dtype=np.float32)
    idx = np.asarray(to_index).astype(np.int64).ravel()
    N = int(dim_size)
    Wm = np.asarray(W, dtype=np.float32)
    bv = np.asarray(b, dtype=np.float32).ravel()
    E, D_in = X.shape
    assert D_in == D and Wm.shape == (D, D)

    NS = -(-N // N_CORES)                      # nodes per core
    order = np.argsort(idx, kind="stable")
    sidx = idx[order]
    bounds = np.searchsorted(sidx, np.arange(N_CORES + 1) * NS)

    packs = []
    for c in range(N_CORES):
        lo, hi = int(bounds[c]), int(bounds[c + 1])
        li = sidx[lo:hi] - c * NS
        eid = order[lo:hi]
        starts, ends, soe, s2n = _pack_core(li)
        packs.append((li, eid, starts, ends, soe, s2n))

    NCp = -(-max(len(p[2]) for p in packs) // HB) * HB

    iota_dev = np.ascontiguousarray(
        np.broadcast_to(np.arange(S, dtype=np.float32), (P, S))
    ).astype(BF16)
    wdup_dev = np.ascontiguousarray(
        np.concatenate([Wm.T, Wm.T], axis=0)
    ).astype(BF16)

    in_maps = []
    for c in range(N_CORES):
        li, eid, starts, ends, soe, s2n = packs[c]
        X_dev, li_dev = _prep_core(X, eid, li, soe, starts, ends, NCp)
        in_maps.append(
            {"xdev": X_dev, "lidev": li_dev, "iota": iota_dev, "wdup": wdup_dev}
        )

    nc = _build_bass(NCp)
    last_exc = None
    for attempt in range(3):
        try:
            res = run_bass_kernel_spmd(
                nc, in_maps, core_ids=list(range(N_CORES)), trace=_trace
            )
            break
        except Exception as exc:  # transient NRT device errors: retry
            last_exc = exc
            import time as _time

            _time.sleep(2.0)
    else:
        raise last_exc
    _LAST_PERF.clear()
    _LAST_PERF.update(
        exec_time_ns=res.exec_time_ns,
        mean_exec_time_ns=res.mean_exec_time_ns,
        trace=res.instructions_and_trace[1] if res.instructions_and_trace else None,
    )

    out = np.empty((N, D), np.float32)
    for c in range(N_CORES):
        n0 = c * NS
        n1 = min(N, (c + 1) * NS)
        s2n = packs[c][5]
        s2n_pad = np.full((NCp, S), -1, np.int64)
        s2n_pad[: len(s2n)] = s2n
        # device column order: per bank, even (A) chunks then odd (B) chunks
        bank_sizes, bank_cstart = _bank_layout(NCp)
        order = []
        for b0, bs in zip(bank_cstart, bank_sizes):
            order.extend(range(b0, b0 + bs, 2))
            order.extend(range(b0 + 1, b0 + bs, 2))
        flat = s2n_pad[np.asarray(order)].ravel()
        rows = res.results[c]["out"].astype(np.float32).T  # [NCp*S, D]
        valid = flat >= 0
        acc = np.zeros((n1 - n0, D), np.float32)
        np.add.at(acc, flat[valid], rows[valid])
        out[n0:n1] = acc + bv
    return out


# revision 38
# speedup vs baseline: 1.3656x; 1.3656x over previous
"""GNN message-passing (scatter-mean + Linear) kernel for 8 Trainium2 NeuronCores.

reference:
    sums   = segment_sum(from_tensor, to_index, N)        # [N, 64]
    counts = segment_sum(ones, to_index, N)               # [N, 1]
    out    = (sums / max(counts, 1)) @ W.T + b            # [N, 64]

Sharding: edges are partitioned across the 8 cores BY DESTINATION NODE RANGE
(each core owns a contiguous block of N/8 nodes and receives the edges
targeting them), so no cross-core reduction is needed.

Device algorithm (per core): edges sorted by destination are cut into
128-edge chunks; a chunk touches at most S=8 distinct nodes (the rare chunk
that would touch more is cut early), so the chunk one-hot H[e, s] is only
[128, 8] — built on VectorE with a batched is_equal.  Chunks are processed
in PAIRS by a single TensorE matmul: the stationary is the two chunks'
features side by side ([128, 128] bf16) and the rhs is [H_A | H_B]
([128, 16]); the off-diagonal blocks of the [128, 16] PSUM output are
garbage and simply never read.  This halves the PE instruction count (the
real-HW bottleneck: per-matmul fixed cost + LdWeights).  64 chunks share a
[128, 512] PSUM bank; two strided ActE copies per bank extract chunk-A rows
(partitions 0:64) and chunk-B rows (partitions 64:128) into a shared SBUF
sums tile (bf16) with A on partitions 0:64 and B on 64:128.  Features are
PRESCALED on the host by 1/count[node], so slot sums are already means.
The final Linear runs per bank as two matmuls with W.T stationary at base
partition 0 (A) and 64 (B).  The bias is NOT applied on device: the host
gather computes out[n] = sum(slot rows of n) + b, which uniformly handles
normal nodes, nodes split across a chunk boundary, and empty nodes.
"""

import dataclasses

import ml_dtypes
import numpy as np

N_CORES = 8
P = 128          # SBUF partitions == edges per chunk == matmul contraction dim
S = 8            # one-hot width: max distinct nodes per chunk
HB = 32          # chunks per batched H build
TC = 128         # chunks per X-stream DMA tile
BK = 64          # chunks per PSUM bank (BK*S == 512 f32 == one 2KB bank)
PB = BK // 2     # pairs per bank
D = 64           # feature dim (in == out)

BF16 = ml_dtypes.bfloat16
FP8 = ml_dtypes.float8_e3m4   # TRN FP8_EXP3: 1-3-4, max normal +-15.5
FP8_MAX = 15.5


def _pack_core(li):
    """Cut the sorted local node ids into 128-edge chunks, each touching at
    most S distinct nodes (cut early otherwise).  Returns (starts, ends,
    slot_of_edge, slot2node[NC, S])."""
    Ec = len(li)
    starts, ends, s2n = [], [], []
    slot_of_edge = np.empty(Ec, np.int64)
    pos = 0
    while pos < Ec:
        end = min(pos + P, Ec)
        seg = li[pos:end]
        u, first = np.unique(seg, return_index=True)
        if len(u) > S:
            end = pos + int(first[S])
            u = u[:S]
            seg = li[pos:end]
        slot_of_edge[pos:end] = np.searchsorted(u, seg)
        row = np.full(S, -1, np.int64)
        row[: len(u)] = u
        starts.append(pos)
        ends.append(end)
        s2n.append(row)
        pos = end
    return (
        np.asarray(starts),
        np.asarray(ends),
        slot_of_edge,
        np.asarray(s2n),
    )


def _prep_core(X, eid, li, slot_of_edge, starts, ends, NCp, scale):
    """Build one core's device arrays: X_dev [P, NCp*D] fp8-e3m4 (prescaled
    by scale/count), li_dev [P, NCp] bf16 (slot ids)."""
    Ec = len(li)
    counts = np.bincount(li)
    recip = (scale / counts[li]).astype(np.float32)

    NCc = len(starts)
    chunk_of = np.repeat(np.arange(NCc), ends - starts)
    row = np.arange(Ec) - starts[chunk_of] + chunk_of * P

    Xg = np.zeros((NCp * P, D), np.float32)
    Xg[row] = np.clip(X[eid] * recip[:, None], -FP8_MAX, FP8_MAX)
    X_dev = np.ascontiguousarray(
        Xg.astype(FP8).reshape(NCp, P, D).transpose(1, 0, 2)
    ).reshape(P, NCp * D)

    lis = np.zeros(NCp * P, np.float32)
    lis[row] = slot_of_edge
    li_dev = np.ascontiguousarray(lis.reshape(NCp, P).T.astype(BF16))
    return X_dev, li_dev


def _bank_layout(NCp):
    """Full BK-chunk banks, with the trailing chunks split into small banks
    so the end-of-run extract->Linear->copy->flush chain is short (it sits
    entirely after the last X byte lands)."""
    rem = NCp - BK * max(0, (NCp - BK) // BK)
    bank_sizes = [BK] * ((NCp - rem) // BK)
    for t in (rem - 32, 16, 8, 8) if rem >= 64 else (rem,):
        if t > 0:
            bank_sizes.append(t)
    bank_cstart = [0]
    for bs in bank_sizes:
        bank_cstart.append(bank_cstart[-1] + bs)
    assert bank_cstart[-1] == NCp
    return bank_sizes, bank_cstart


def _build_bass(NCp):
    import concourse.bacc as bacc
    import concourse.mybir as mybir
    import concourse.tile as tile

    f32 = mybir.dt.float32
    bf16 = mybir.dt.bfloat16
    f8 = mybir.dt.float8e3

    NP = NCp // 2           # chunk pairs
    HC = NCp * S // 2       # columns per half (A / B) of sums and out
    assert NCp % HB == 0
    bank_sizes, bank_cstart = _bank_layout(NCp)
    NB = len(bank_sizes)
    bank_of_chunk = np.repeat(np.arange(NB), bank_sizes)

    nc = bacc.Bacc("TRN2", target_bir_lowering=False)
    X_t = nc.dram_tensor("xdev", [P, NCp * D], f8, kind="ExternalInput")
    li_t = nc.dram_tensor("lidev", [P, NCp], bf16, kind="ExternalInput")
    iota_t = nc.dram_tensor("iota", [P, S], bf16, kind="ExternalInput")
    w_t = nc.dram_tensor("wdup", [P, D], bf16, kind="ExternalInput")
    out_t = nc.dram_tensor("out", [D, NCp * S], bf16, kind="ExternalOutput")

    with tile.TileContext(nc) as tc:
        with (
            tc.tile_pool(name="const", bufs=1) as cp,
            tc.tile_pool(name="xin", bufs=7) as xp,
            tc.tile_pool(name="hp", bufs=4) as hp,
            tc.tile_pool(name="big", bufs=1) as bigp,
            tc.tile_pool(name="ps", bufs=4, space="PSUM") as pp,
            tc.tile_pool(name="ps2", bufs=4, space="PSUM") as pp2,
        ):
            # (tile schedule is needed below to place the lirel split safely)
            iota = cp.tile([P, S], bf16)
            nc.scalar.dma_start(out=iota[:], in_=iota_t[:, :])
            wdup = cp.tile([P, D], bf16)
            nc.scalar.dma_start(out=wdup[:], in_=w_t[:, :])

            # A-chunk sums on partitions 0:64, B-chunk sums on 64:128,
            # sharing columns (bank bi -> cols bi*256 .. bi*256+256)
            sums = bigp.tile([P, HC], bf16)
            outsb = bigp.tile([D, NCp * S], bf16)

            # moderate first/last tiles: keep per-partition DMA runs >= 4KB
            # (32 chunks) for full DMA rate while shrinking the PE backlog
            # that remains after the last X byte lands
            head = [32, 64] if NCp >= 256 else []
            tail = [64, 32, 32] if NCp >= 256 else []
            mid = NCp - sum(head) - sum(tail)
            sizes = (
                head
                + [TC] * (mid // TC)
                + ([mid % TC] if mid % TC else [])
                + tail
            )
            tiles = []
            base = 0
            for size in sizes:
                tiles.append((base, size))
                base += size
            assert base == NCp
            tile_of_chunk = {}
            for t, (b0, sz) in enumerate(tiles):
                for j in range(b0, b0 + sz):
                    tile_of_chunk[j] = t

            # lirel rides the fast sync queue AHEAD of the X tiles: the first
            # H build gates the whole PE pipeline start.  Only the slice the
            # first tile's H builds need goes up front; the rest is issued
            # right after the second tile's DMA (before any H build reads it).
            lsplit = tiles[1][0] if len(tiles) > 1 else NCp
            lirel = cp.tile([P, NCp], bf16)
            nc.sync.dma_start(out=lirel[:, :lsplit], in_=li_t[:, :lsplit])

            def emit_bank_extract(bi):
                # strided reads: pair p of this bank wrote [128, 16] at col
                # p*16; A slots are cols 0:8 (rows 0:64), B are 8:16 (64:128)
                np_ = bank_sizes[bi] // 2          # pairs in this bank
                c0_ = bank_cstart[bi] * S // 2     # sums col base
                tA = bank[0:64, :]
                inA = dataclasses.replace(
                    tA, ap=[tA.ap[0], [2 * S, np_], [1, S]]
                )
                outA = sums[0:64, c0_ : c0_ + np_ * S]
                nc.scalar.copy(
                    out=outA.rearrange("p (c w) -> p c w", w=S), in_=inA
                )
                tB = bank[64:128, S : np_ * 2 * S]
                inB = dataclasses.replace(
                    tB, ap=[tB.ap[0], [2 * S, np_], [1, S]]
                )
                outB = sums[64:128, c0_ : c0_ + np_ * S]
                nc.scalar.copy(
                    out=outB.rearrange("p (c w) -> p c w", w=S), in_=inB
                )

            def emit_final(bi):
                # per-bank Linear: A on partitions 0:64, B on 64:128; outsb
                # is bank-contiguous [A(w) | B(w)] so flushes are single DMAs
                w = bank_sizes[bi] * S // 2
                c0_ = bank_cstart[bi] * S // 2
                base = bank_cstart[bi] * S
                for half, p0 in ((0, 0), (1, 64)):
                    o2 = pp2.tile([D, w], f32)
                    nc.tensor.matmul(
                        o2[:],
                        lhsT=wdup[p0 : p0 + 64, :],
                        rhs=sums[p0 : p0 + 64, c0_ : c0_ + w],
                        start=True,
                        stop=True,
                    )
                    nc.scalar.copy(
                        out=outsb[:, base + half * w : base + (half + 1) * w],
                        in_=o2[:],
                    )

            def emit_out_dma(b0, b1, eng):
                c0_ = bank_cstart[b0] * S
                c1_ = bank_cstart[b1] * S
                eng.dma_start(
                    out=out_t[:, c0_:c1_], in_=outsb[:, c0_:c1_]
                )

            xt = h = bank = None
            xt_base = 0
            fin_done = 0
            dma_done = 0
            for q in range(NP):
                c0 = 2 * q
                t = tile_of_chunk[c0]
                if c0 == tiles[t][0]:
                    b0, sz = tiles[t]
                    if t == 1 and lsplit < NCp:
                        nc.sync.dma_start(
                            out=lirel[:, lsplit:], in_=li_t[:, lsplit:]
                        )
                    xt = xp.tile([P, TC * D], f8, name="xt")
                    nc.sync.dma_start(
                        out=xt[:, : sz * D],
                        in_=X_t[:, b0 * D : (b0 + sz) * D],
                    )
                    xt_base = b0
                if c0 % HB == 0:
                    h = hp.tile([P, HB * S], f8)
                    in0 = lirel[:, c0 : c0 + HB].to_broadcast([P, HB, S])
                    ia = iota[:, :]
                    in1 = dataclasses.replace(ia, ap=[ia.ap[0], [0, HB], [1, S]])
                    nc.vector.tensor_tensor(
                        out=h[:].rearrange("p (c w) -> p c w", w=S),
                        in0=in1,
                        in1=in0,
                        op=mybir.AluOpType.is_equal,
                    )
                bi = bank_of_chunk[c0]
                bq = (c0 - bank_cstart[bi]) // 2   # pair index within bank
                if bq == 0:
                    bank = pp.tile([P, bank_sizes[bi] * S], f32)
                nc.tensor.matmul(
                    bank[:, bq * 2 * S : (bq + 1) * 2 * S],
                    lhsT=xt[:, (c0 - xt_base) * D : (c0 - xt_base + 2) * D],
                    rhs=h[:, (c0 % HB) * S : ((c0 % HB) + 2) * S],
                    start=True,
                    stop=True,
                )
                if c0 == bank_cstart[bi + 1] - 2:
                    emit_bank_extract(bi)
                # run the Linear for bank bi one bank late so the PE never
                # head-of-line blocks on the ActE bank extraction
                if bq == min(8, bank_sizes[bi] // 4) and bi >= 1:
                    emit_final(fin_done)
                    fin_done += 1
                    if fin_done - dma_done >= 6:
                        emit_out_dma(dma_done, fin_done, nc.scalar)
                        dma_done = fin_done
            while fin_done < NB:
                emit_final(fin_done)
                fin_done += 1
            # tail flush rides the sync queue (full DMA rate, right after
            # the last X tile)
            emit_out_dma(dma_done, NB, nc.sync)
    nc.compile()
    return nc


_LAST_PERF = {}  # filled by kernel(): exec_time_ns etc (read by test.py)


def kernel(from_tensor, to_index, dim_size, W, b, _trace=False):
    from concourse.bass_utils import run_bass_kernel_spmd

    X = np.ascontiguousarray(np.asarray(from_tensor), dtype=np.float32)
    idx = np.asarray(to_index).astype(np.int64).ravel()
    N = int(dim_size)
    Wm = np.asarray(W, dtype=np.float32)
    bv = np.asarray(b, dtype=np.float32).ravel()
    E, D_in = X.shape
    assert D_in == D and Wm.shape == (D, D)

    NS = -(-N // N_CORES)                      # nodes per core
    order = np.argsort(idx, kind="stable")
    sidx = idx[order]
    bounds = np.searchsorted(sidx, np.arange(N_CORES + 1) * NS)

    packs = []
    for c in range(N_CORES):
        lo, hi = int(bounds[c]), int(bounds[c + 1])
        li = sidx[lo:hi] - c * NS
        eid = order[lo:hi]
        starts, ends, soe, s2n = _pack_core(li)
        packs.append((li, eid, starts, ends, soe, s2n))

    NCp = -(-max(len(p[2]) for p in packs) // HB) * HB

    # global prescale: X rides in fp8-e3m4 as x*scale/count; 1/scale is
    # folded into the Linear weights.  scale fills the e3m4 range without
    # clipping (~32 for this distribution).
    maxv = 0.0
    for c in range(N_CORES):
        li, eid = packs[c][0], packs[c][1]
        counts = np.bincount(li)
        m = np.abs(X[eid]).max(axis=1) / counts[li]
        maxv = max(maxv, float(m.max()) if len(m) else 0.0)
    scale = min(48.0, FP8_MAX / max(maxv, 1e-30))

    iota_dev = np.ascontiguousarray(
        np.broadcast_to(np.arange(S, dtype=np.float32), (P, S))
    ).astype(BF16)
    wt = Wm.T / scale
    wdup_dev = np.ascontiguousarray(np.concatenate([wt, wt], axis=0)).astype(
        BF16
    )

    in_maps = []
    for c in range(N_CORES):
        li, eid, starts, ends, soe, s2n = packs[c]
        X_dev, li_dev = _prep_core(X, eid, li, soe, starts, ends, NCp, scale)
        in_maps.append(
            {"xdev": X_dev, "lidev": li_dev, "iota": iota_dev, "wdup": wdup_dev}
        )

    nc = _build_bass(NCp)
    last_exc = None
    for attempt in range(3):
        try:
            res = run_bass_kernel_spmd(
                nc, in_maps, core_ids=list(range(N_CORES)), trace=_trace
            )
            break
        except Exception as exc:  # transient NRT device errors: retry
            last_exc = exc
            import time as _time

            _time.sleep(2.0)
    else:
        raise last_exc
    _LAST_PERF.clear()
    _LAST_PERF.update(
        exec_time_ns=res.exec_time_ns,
        mean_exec_time_ns=res.mean_exec_time_ns,
        trace=res.instructions_and_trace[1] if res.instructions_and_trace else None,
    )

    out = np.empty((N, D), np.float32)
    for c in range(N_CORES):
        n0 = c * NS
        n1 = min(N, (c + 1) * NS)
        s2n = packs[c][5]
        s2n_pad = np.full((NCp, S), -1, np.int64)
        s2n_pad[: len(s2n)] = s2n
        # device column order: per bank, even (A) chunks then odd (B) chunks
        bank_sizes, bank_cstart = _bank_layout(NCp)
        order = []
        for b0, bs in zip(bank_cstart, bank_sizes):
            order.extend(range(b0, b0 + bs, 2))
            order.extend(range(b0 + 1, b0 + bs, 2))
        flat = s2n_pad[np.asarray(order)].ravel()
        rows = res.results[c]["out"].astype(np.float32).T  # [NCp*S, D]
        valid = flat >= 0
        acc = np.zeros((n1 - n0, D), np.float32)
        np.add.at(acc, flat[valid], rows[valid])
        out[n0:n1] = acc + bv
    return out


# revision 42
# speedup vs baseline: 1.4680x; 1.0750x over previous
"""GNN message-passing (scatter-mean + Linear) kernel for 8 Trainium2 NeuronCores.

reference:
    sums   = segment_sum(from_tensor, to_index, N)        # [N, 64]
    counts = segment_sum(ones, to_index, N)               # [N, 1]
    out    = (sums / max(counts, 1)) @ W.T + b            # [N, 64]

Sharding: edges are partitioned across the 8 cores BY DESTINATION NODE RANGE
(each core owns a contiguous block of N/8 nodes and receives the edges
targeting them), so no cross-core reduction is needed.

Device algorithm (per core): edges sorted by destination are cut into
128-edge chunks; a chunk touches at most S=8 distinct nodes (the rare chunk
that would touch more is cut early), so the chunk one-hot H[e, s] is only
[128, 8] — built on VectorE with a batched is_equal.  Chunks are processed
in PAIRS by a single TensorE matmul: the stationary is the two chunks'
features side by side ([128, 128] bf16) and the rhs is [H_A | H_B]
([128, 16]); the off-diagonal blocks of the [128, 16] PSUM output are
garbage and simply never read.  This halves the PE instruction count (the
real-HW bottleneck: per-matmul fixed cost + LdWeights).  64 chunks share a
[128, 512] PSUM bank; two strided ActE copies per bank extract chunk-A rows
(partitions 0:64) and chunk-B rows (partitions 64:128) into a shared SBUF
sums tile (bf16) with A on partitions 0:64 and B on 64:128.  Features are
PRESCALED on the host by 1/count[node], so slot sums are already means.
The final Linear runs per bank as two matmuls with W.T stationary at base
partition 0 (A) and 64 (B).  The bias is NOT applied on device: the host
gather computes out[n] = sum(slot rows of n) + b, which uniformly handles
normal nodes, nodes split across a chunk boundary, and empty nodes.
"""

import dataclasses

import ml_dtypes
import numpy as np

N_CORES = 8
P = 128          # SBUF partitions == edges per chunk == matmul contraction dim
S = 8            # one-hot width: max distinct nodes per chunk
HB = 32          # chunks per batched H build
TC = 128         # chunks per X-stream DMA tile
BK = 64          # chunks per PSUM bank (BK*S == 512 f32 == one 2KB bank)
PB = BK // 2     # pairs per bank
D = 64           # feature dim (in == out)

BF16 = ml_dtypes.bfloat16
FP8 = ml_dtypes.float8_e3m4   # TRN FP8_EXP3: 1-3-4, max normal +-15.5
FP8_MAX = 15.5


def _pack_core(li):
    """Cut the sorted local node ids into 128-edge chunks, each touching at
    most S distinct nodes (cut early otherwise).  Returns (starts, ends,
    slot_of_edge, slot2node[NC, S])."""
    Ec = len(li)
    starts, ends, s2n = [], [], []
    slot_of_edge = np.empty(Ec, np.int64)
    pos = 0
    while pos < Ec:
        end = min(pos + P, Ec)
        seg = li[pos:end]
        u, first = np.unique(seg, return_index=True)
        if len(u) > S:
            end = pos + int(first[S])
            u = u[:S]
            seg = li[pos:end]
        slot_of_edge[pos:end] = np.searchsorted(u, seg)
        row = np.full(S, -1, np.int64)
        row[: len(u)] = u
        starts.append(pos)
        ends.append(end)
        s2n.append(row)
        pos = end
    return (
        np.asarray(starts),
        np.asarray(ends),
        slot_of_edge,
        np.asarray(s2n),
    )


def _prep_core(X, eid, li, slot_of_edge, starts, ends, NCp, scale):
    """Build one core's device arrays: X_dev [P, NCp*D] fp8-e3m4 (prescaled
    by scale/count), li_dev [P, NCp] bf16 (slot ids)."""
    Ec = len(li)
    counts = np.bincount(li)
    recip = (scale / counts[li]).astype(np.float32)

    NCc = len(starts)
    chunk_of = np.repeat(np.arange(NCc), ends - starts)
    row = np.arange(Ec) - starts[chunk_of] + chunk_of * P

    Xg = np.zeros((NCp * P, D), np.float32)
    Xg[row] = np.clip(X[eid] * recip[:, None], -FP8_MAX, FP8_MAX)
    X_dev = np.ascontiguousarray(
        Xg.astype(FP8).reshape(NCp, P, D).transpose(1, 0, 2)
    ).reshape(P, NCp * D)

    lis = np.zeros(NCp * P, np.float32)
    lis[row] = slot_of_edge
    li_dev = np.ascontiguousarray(lis.reshape(NCp, P).T.astype(BF16))
    return X_dev, li_dev


def _bank_layout(NCp):
    """Full BK-chunk banks, with the trailing chunks split into small banks
    so the end-of-run extract->Linear->copy->flush chain is short (it sits
    entirely after the last X byte lands)."""
    rem = NCp - BK * max(0, (NCp - BK) // BK)
    bank_sizes = [BK] * ((NCp - rem) // BK)
    for t in (rem - 32, 16, 8, 8) if rem >= 64 else (rem,):
        if t > 0:
            bank_sizes.append(t)
    bank_cstart = [0]
    for bs in bank_sizes:
        bank_cstart.append(bank_cstart[-1] + bs)
    assert bank_cstart[-1] == NCp
    return bank_sizes, bank_cstart


def _build_bass(NCp):
    import concourse.bacc as bacc
    import concourse.mybir as mybir
    import concourse.tile as tile

    f32 = mybir.dt.float32
    bf16 = mybir.dt.bfloat16
    f8 = mybir.dt.float8e3

    NP = NCp // 2           # chunk pairs
    HC = NCp * S // 2       # columns per half (A / B) of sums and out
    assert NCp % HB == 0
    bank_sizes, bank_cstart = _bank_layout(NCp)
    NB = len(bank_sizes)
    bank_of_chunk = np.repeat(np.arange(NB), bank_sizes)
    # final-Linear groups: pairs of consecutive banks (one stationary load
    # serves up to 512 sums columns)
    groups = [(b, min(b + 1, NB - 1)) for b in range(0, NB, 2)]

    nc = bacc.Bacc("TRN2", target_bir_lowering=False)
    X_t = nc.dram_tensor("xdev", [P, NCp * D], f8, kind="ExternalInput")
    li_t = nc.dram_tensor("lidev", [P, NCp], bf16, kind="ExternalInput")
    iota_t = nc.dram_tensor("iota", [P, S], bf16, kind="ExternalInput")
    w_t = nc.dram_tensor("wdup", [P, D], bf16, kind="ExternalInput")
    out_t = nc.dram_tensor("out", [D, NCp * S], bf16, kind="ExternalOutput")

    with tile.TileContext(nc) as tc:
        with (
            tc.tile_pool(name="const", bufs=1) as cp,
            tc.tile_pool(name="xin", bufs=7) as xp,
            tc.tile_pool(name="hp", bufs=4) as hp,
            tc.tile_pool(name="big", bufs=1) as bigp,
            tc.tile_pool(name="ps", bufs=4, space="PSUM") as pp,
            tc.tile_pool(name="ps2", bufs=4, space="PSUM") as pp2,
        ):
            # (tile schedule is needed below to place the lirel split safely)
            iota = cp.tile([P, S], bf16)
            nc.scalar.dma_start(out=iota[:], in_=iota_t[:, :])
            wdup = cp.tile([P, D], bf16)
            nc.scalar.dma_start(out=wdup[:], in_=w_t[:, :])

            # A-chunk sums on partitions 0:64, B-chunk sums on 64:128,
            # sharing columns (bank bi -> cols bi*256 .. bi*256+256)
            sums = bigp.tile([P, HC], bf16)
            outsb = bigp.tile([D, NCp * S], bf16)

            # moderate first/last tiles: keep per-partition DMA runs >= 4KB
            # (32 chunks) for full DMA rate while shrinking the PE backlog
            # that remains after the last X byte lands
            head = [32, 64] if NCp >= 256 else []
            tail = [64, 32, 32] if NCp >= 256 else []
            mid = NCp - sum(head) - sum(tail)
            sizes = (
                head
                + [TC] * (mid // TC)
                + ([mid % TC] if mid % TC else [])
                + tail
            )
            tiles = []
            base = 0
            for size in sizes:
                tiles.append((base, size))
                base += size
            assert base == NCp
            tile_of_chunk = {}
            for t, (b0, sz) in enumerate(tiles):
                for j in range(b0, b0 + sz):
                    tile_of_chunk[j] = t

            # lirel rides the fast sync queue AHEAD of the X tiles: the first
            # H build gates the whole PE pipeline start.  Only the slice the
            # first tile's H builds need goes up front; the rest is issued
            # right after the second tile's DMA (before any H build reads it).
            lsplit = tiles[1][0] if len(tiles) > 1 else NCp
            lirel = cp.tile([P, NCp], bf16)
            nc.sync.dma_start(out=lirel[:, :lsplit], in_=li_t[:, :lsplit])

            def emit_bank_extract(bi):
                # strided reads: pair p of this bank wrote [128, 16] at col
                # p*16; A slots are cols 0:8 (rows 0:64), B are 8:16 (64:128)
                np_ = bank_sizes[bi] // 2          # pairs in this bank
                c0_ = bank_cstart[bi] * S // 2     # sums col base
                tA = bank[0:64, :]
                inA = dataclasses.replace(
                    tA, ap=[tA.ap[0], [2 * S, np_], [1, S]]
                )
                outA = sums[0:64, c0_ : c0_ + np_ * S]
                nc.scalar.copy(
                    out=outA.rearrange("p (c w) -> p c w", w=S), in_=inA
                )
                tB = bank[64:128, S : np_ * 2 * S]
                inB = dataclasses.replace(
                    tB, ap=[tB.ap[0], [2 * S, np_], [1, S]]
                )
                outB = sums[64:128, c0_ : c0_ + np_ * S]
                nc.scalar.copy(
                    out=outB.rearrange("p (c w) -> p c w", w=S), in_=inB
                )

            def emit_final(gi):
                # per-group (2 banks) Linear: A on partitions 0:64, B on
                # 64:128; outsb is group-contiguous [A(w) | B(w)] so flushes
                # are single DMAs
                b0_, b1_ = groups[gi]
                w = (bank_cstart[b1_ + 1] - bank_cstart[b0_]) * S // 2
                c0_ = bank_cstart[b0_] * S // 2
                base = bank_cstart[b0_] * S
                for half, p0 in ((0, 0), (1, 64)):
                    o2 = pp2.tile([D, w], f32)
                    nc.tensor.matmul(
                        o2[:],
                        lhsT=wdup[p0 : p0 + 64, :],
                        rhs=sums[p0 : p0 + 64, c0_ : c0_ + w],
                        start=True,
                        stop=True,
                    )
                    nc.scalar.copy(
                        out=outsb[:, base + half * w : base + (half + 1) * w],
                        in_=o2[:],
                    )

            def emit_out_dma(g0, g1, eng):
                c0_ = bank_cstart[groups[g0][0]] * S
                c1_ = bank_cstart[groups[g1 - 1][1] + 1] * S
                eng.dma_start(
                    out=out_t[:, c0_:c1_], in_=outsb[:, c0_:c1_]
                )

            xt = h = bank = None
            xt_base = 0
            fin_done = 0
            dma_done = 0
            for q in range(NP):
                c0 = 2 * q
                t = tile_of_chunk[c0]
                if c0 == tiles[t][0]:
                    b0, sz = tiles[t]
                    if t == 1 and lsplit < NCp:
                        nc.sync.dma_start(
                            out=lirel[:, lsplit:], in_=li_t[:, lsplit:]
                        )
                    xt = xp.tile([P, TC * D], f8, name="xt")
                    nc.sync.dma_start(
                        out=xt[:, : sz * D],
                        in_=X_t[:, b0 * D : (b0 + sz) * D],
                    )
                    xt_base = b0
                if c0 % HB == 0:
                    h = hp.tile([P, HB * S], f8)
                    in0 = lirel[:, c0 : c0 + HB].to_broadcast([P, HB, S])
                    ia = iota[:, :]
                    in1 = dataclasses.replace(ia, ap=[ia.ap[0], [0, HB], [1, S]])
                    nc.vector.tensor_tensor(
                        out=h[:].rearrange("p (c w) -> p c w", w=S),
                        in0=in1,
                        in1=in0,
                        op=mybir.AluOpType.is_equal,
                    )
                bi = bank_of_chunk[c0]
                bq = (c0 - bank_cstart[bi]) // 2   # pair index within bank
                if bq == 0:
                    bank = pp.tile([P, bank_sizes[bi] * S], f32)
                nc.tensor.matmul(
                    bank[:, bq * 2 * S : (bq + 1) * 2 * S],
                    lhsT=xt[:, (c0 - xt_base) * D : (c0 - xt_base + 2) * D],
                    rhs=h[:, (c0 % HB) * S : ((c0 % HB) + 2) * S],
                    start=True,
                    stop=True,
                )
                if c0 == bank_cstart[bi + 1] - 2:
                    emit_bank_extract(bi)
                # run the Linear for a group once its banks are extracted and
                # the loop has moved past them (so the PE never head-of-line
                # blocks on the ActE bank extraction)
                if bq == min(8, bank_sizes[bi] // 4):
                    while fin_done < len(groups) and groups[fin_done][1] < bi:
                        emit_final(fin_done)
                        fin_done += 1
                        if fin_done - dma_done >= 3:
                            emit_out_dma(dma_done, fin_done, nc.scalar)
                            dma_done = fin_done
            while fin_done < len(groups):
                emit_final(fin_done)
                fin_done += 1
            # tail flush rides the sync queue (full DMA rate, right after
            # the last X tile)
            emit_out_dma(dma_done, len(groups), nc.sync)
    nc.compile()
    return nc


_LAST_PERF = {}  # filled by kernel(): exec_time_ns etc (read by test.py)


def kernel(from_tensor, to_index, dim_size, W, b, _trace=False):
    from concourse.bass_utils import run_bass_kernel_spmd

    X = np.ascontiguousarray(np.asarray(from_tensor), dtype=np.float32)
    idx = np.asarray(to_index).astype(np.int64).ravel()
    N = int(dim_size)
    Wm = np.asarray(W, dtype=np.float32)
    bv = np.asarray(b, dtype=np.float32).ravel()
    E, D_in = X.shape
    assert D_in == D and Wm.shape == (D, D)

    NS = -(-N // N_CORES)                      # nodes per core
    order = np.argsort(idx, kind="stable")
    sidx = idx[order]
    bounds = np.searchsorted(sidx, np.arange(N_CORES + 1) * NS)

    packs = []
    for c in range(N_CORES):
        lo, hi = int(bounds[c]), int(bounds[c + 1])
        li = sidx[lo:hi] - c * NS
        eid = order[lo:hi]
        starts, ends, soe, s2n = _pack_core(li)
        packs.append((li, eid, starts, ends, soe, s2n))

    NCp = -(-max(len(p[2]) for p in packs) // HB) * HB

    # global prescale: X rides in fp8-e3m4 as x*scale/count; 1/scale is
    # folded into the Linear weights.  scale fills the e3m4 range without
    # clipping (~32 for this distribution).
    maxv = 0.0
    for c in range(N_CORES):
        li, eid = packs[c][0], packs[c][1]
        counts = np.bincount(li)
        m = np.abs(X[eid]).max(axis=1) / counts[li]
        maxv = max(maxv, float(m.max()) if len(m) else 0.0)
    scale = min(48.0, FP8_MAX / max(maxv, 1e-30))

    iota_dev = np.ascontiguousarray(
        np.broadcast_to(np.arange(S, dtype=np.float32), (P, S))
    ).astype(BF16)
    wt = Wm.T / scale
    wdup_dev = np.ascontiguousarray(np.concatenate([wt, wt], axis=0)).astype(
        BF16
    )

    in_maps = []
    for c in range(N_CORES):
        li, eid, starts, ends, soe, s2n = packs[c]
        X_dev, li_dev = _prep_core(X, eid, li, soe, starts, ends, NCp, scale)
        in_maps.append(
            {"xdev": X_dev, "lidev": li_dev, "iota": iota_dev, "wdup": wdup_dev}
        )

    nc = _build_bass(NCp)
    last_exc = None
    for attempt in range(3):
        try:
            res = run_bass_kernel_spmd(
                nc, in_maps, core_ids=list(range(N_CORES)), trace=_trace
            )
            break
        except Exception as exc:  # transient NRT device errors: retry
            last_exc = exc
            import time as _time

            _time.sleep(2.0)
    else:
        raise last_exc
    _LAST_PERF.clear()
    _LAST_PERF.update(
        exec_time_ns=res.exec_time_ns,
        mean_exec_time_ns=res.mean_exec_time_ns,
        trace=res.instructions_and_trace[1] if res.instructions_and_trace else None,
    )

    out = np.empty((N, D), np.float32)
    for c in range(N_CORES):
        n0 = c * NS
        n1 = min(N, (c + 1) * NS)
        s2n = packs[c][5]
        s2n_pad = np.full((NCp, S), -1, np.int64)
        s2n_pad[: len(s2n)] = s2n
        # device column order: per final-Linear group (2 banks), even (A)
        # chunks of its banks then odd (B) chunks
        bank_sizes, bank_cstart = _bank_layout(NCp)
        NB = len(bank_sizes)
        order = []
        for g0 in range(0, NB, 2):
            g1 = min(g0 + 1, NB - 1)
            for par in (0, 1):
                for bb in range(g0, g1 + 1):
                    order.extend(
                        range(bank_cstart[bb] + par, bank_cstart[bb + 1], 2)
                    )
        flat = s2n_pad[np.asarray(order)].ravel()
        rows = res.results[c]["out"].astype(np.float32).T  # [NCp*S, D]
        valid = flat >= 0
        acc = np.zeros((n1 - n0, D), np.float32)
        np.add.at(acc, flat[valid], rows[valid])
        out[n0:n1] = acc + bv
    return out
